# revision 7
# baseline (speedup 1.0000x reference)
"""Multi-head self-attention (B=4, S=2048, D=1024, H=16) on 8 trn2 NeuronCores.

Sharding: batch (4) x head-group (2 groups of 8 heads) -> 8 cores.
Each core computes, for its (batch b, head-group hg):
  Q'^T = (wq_l/8) @ x_b^T            [512, 2048]   (1/sqrt(dk) folded into wq)
  K^T  = wk_l @ x_b^T                [512, 2048]
  V    = x_b @ wv_l^T                [2048, 512]
  per head h (8 local, dk=64), in transposed layout (keys on partitions):
    scoresT[k, q] = K_h @ Q'_h^T     (no max-subtraction: scores ~ N(0,4), exp
                                      of |s|<~12 is safe in fp32/bf16)
    expT = exp(scoresT)              (ScalarE, PSUM->SBUF bf16)
    unnormT[c, q] = V_h^T @ expT     (PE, accumulated over key tiles)
    Z[q] = ones^T @ expT             (PE colsum, same accumulation)
    attnT = unnormT / Z              (broadcast Z via selector-matmul + DVE mul)
  out_partial = attnT^T @ wo_l^T     [2048, 1024]  (row-parallel wo)
Host sums the two partials per batch (the "all-reduce" of row-parallel wo).
"""

import sys
import types

import ml_dtypes
import numpy as np

import bass_rust
import concourse.bass as bass
import concourse.mybir as mybir
import concourse.tile as tile

# ---------------------------------------------------------------- constants
S = 2048          # sequence length
DM = 1024         # model dim
DL = 512          # local (per-core) head dims = 8 heads * 64
DK = 64           # head dim
P = 128
NKT = S // P      # 16 key tiles
NG = DL // P      # 4 head-pairs (c-tiles / dq-tiles)
KD = DM // P      # 8 contraction tiles for projections
NSC = S // 512    # 4 s-chunks for projections
F32 = mybir.dt.float32
BF16 = mybir.dt.bfloat16
BF16_NP = ml_dtypes.bfloat16

N_CORES = 8
CORE_IDS = list(range(N_CORES))


# ------------------------------------------------- walrus sync-wait workaround
def _split_sync_waits(nc, limit=1):
    """This toolchain's walrus codegen rejects instructions carrying more than
    one sync-wait command.  Move excess waits onto dedicated same-engine nops
    inserted immediately before the instruction (sequential waits on the same
    engine queue are semantically identical to multiple waits on one inst)."""
    fn = nc.m.functions[0]
    snapshots = [(bb, list(bb.instructions)) for bb in fn.blocks]
    plans = []
    for _bb, insts in snapshots:
        plan = {}
        for idx, inst in enumerate(insts):
            si = inst.sync_info
            waits = list(si.on_wait) if si and si.on_wait else []
            if len(waits) > limit:
                pre, keep = waits[:-limit], waits[-limit:]
                nops = []
                for w in pre:
                    ni = nc.engines[inst.engine].nop(nofuse=True, hint="wsplit").ins
                    ni.sync_info = bass_rust.SyncInfo(on_wait=[w], on_update=[])
                    nops.append(ni)
                si.on_wait = keep
                plan[idx] = nops
        plans.append(plan)
    # Rebuild every block from its pre-pass snapshot plus insertions; this also
    # drops the fresh nops from wherever bass appended them at creation time.
    for (bb, insts), plan in zip(snapshots, plans):
        out = []
        for idx, inst in enumerate(insts):
            out.extend(plan.get(idx, ()))
            out.append(inst)
        bb.instructions = out


# ---------------------------------------------------------------- the program
def build_nc():
    """Build the SPMD per-core Bass program (identical on all 8 cores)."""
    nc = bass.Bass()

    xT = nc.declare_dram_parameter("xT", [DM, S], BF16, isOutput=False)
    wqT = nc.declare_dram_parameter("wqT", [DM, DL], BF16, isOutput=False)
    wkT = nc.declare_dram_parameter("wkT", [DM, DL], BF16, isOutput=False)
    wvT = nc.declare_dram_parameter("wvT", [DM, DL], BF16, isOutput=False)
    woT = nc.declare_dram_parameter("woT", [DL, DM], BF16, isOutput=False)
    out = nc.declare_dram_parameter("out", [S, DM], F32, isOutput=True)

    with tile.TileContext(nc) as tc:
        with (
            tc.tile_pool(name="big", bufs=1) as big,
            tc.tile_pool(name="expT", bufs=6) as expp,
            tc.tile_pool(name="rc", bufs=2) as rcp,
            tc.tile_pool(name="outsb", bufs=3) as outp,
            tc.tile_pool(name="ps", bufs=2, space="PSUM") as psp,
            tc.tile_pool(name="av", bufs=2, space="PSUM") as avp,
            tc.tile_pool(name="cs", bufs=1, space="PSUM") as csp,
            tc.tile_pool(name="bc", bufs=1, space="PSUM") as bcp,
        ):
            # ---------------- load everything from DRAM
            xT_sb = big.tile([P, KD, S], BF16, tag="xT")
            nc.sync.dma_start(xT_sb[:], xT.rearrange("(kd p) s -> p kd s", p=P))
            w_sb = {}
            for name, dram in (("wq", wqT), ("wk", wkT), ("wv", wvT)):
                w_sb[name] = big.tile([P, KD, DL], BF16, tag=name, name=name)
                nc.sync.dma_start(
                    w_sb[name][:], dram.rearrange("(kd p) m -> p kd m", p=P)
                )
            woT_sb = big.tile([P, NG, DM], BF16, tag="wo")
            nc.sync.dma_start(woT_sb[:], woT.rearrange("(ct p) o -> p ct o", p=P))

            # ---------------- constants
            ones_bf = big.tile([P, 1], BF16, tag="ones")
            nc.vector.memset(ones_bf[:], 1.0)
            # selector for broadcasting Z (rows {0,32}->0:64, {64,96}->64:128)
            sel = big.tile([P, 2 * P], F32, tag="sel")
            nc.vector.memset(sel[:], 0.0)
            nc.vector.memset(sel[0:1, 0:64], 1.0)        # qb0, head A
            nc.vector.memset(sel[64:65, 64:128], 1.0)    # qb0, head B
            nc.vector.memset(sel[32:33, 128:192], 1.0)   # qb1, head A
            nc.vector.memset(sel[96:97, 192:256], 1.0)   # qb1, head B

            # persistent activation tensors
            QT = [big.tile([P, S], BF16, tag=f"QT{g}", name=f"QT{g}") for g in range(NG)]
            KT = [big.tile([P, S], BF16, tag=f"KT{g}", name=f"KT{g}") for g in range(NG)]
            V_st = [big.tile([P, 8, DK + 1], BF16, tag=f"V{st}", name=f"V{st}") for st in range(NKT)]
            for st in range(NKT):
                nc.vector.memset(V_st[st][:, :, DK], 1.0)  # spare ones column
            attn = [big.tile([P, S], BF16, tag=f"attn{g}", name=f"attn{g}") for g in range(NG)]

            # ---------------- projections
            def proj_qk(dst, w, g):
                """dst[:, :] (tile g) = rows [128g,128g+128) of W' @ x^T."""
                for sc in range(NSC):
                    ps = psp.tile([P, 512], F32, tag="ps")
                    for kd in range(KD):
                        nc.tensor.matmul(
                            ps[:],
                            lhsT=w[:, kd, g * P : (g + 1) * P],
                            rhs=xT_sb[:, kd, sc * 512 : (sc + 1) * 512],
                            start=(kd == 0),
                            stop=(kd == KD - 1),
                        )
                    nc.vector.tensor_copy(
                        out=dst[:, sc * 512 : (sc + 1) * 512], in_=ps[:]
                    )

            def proj_v(st):
                ps = psp.tile([P, 512], F32, tag="ps")
                for kd in range(KD):
                    nc.tensor.matmul(
                        ps[:],
                        lhsT=xT_sb[:, kd, st * P : (st + 1) * P],
                        rhs=w_sb["wv"][:, kd, :],
                        start=(kd == 0),
                        stop=(kd == KD - 1),
                    )
                nc.vector.tensor_copy(
                    out=V_st[st][:, :, 0:DK],
                    in_=ps.rearrange("p (h c) -> p h c", c=DK),
                )

            # g=0 first so attention can start early
            proj_qk(KT[0], w_sb["wk"], 0)
            proj_qk(QT[0], w_sb["wq"], 0)
            for st in range(NKT):
                proj_v(st)
            for g in range(1, NG):
                proj_qk(KT[g], w_sb["wk"], g)
                proj_qk(QT[g], w_sb["wq"], g)

            # ---------------- attention
            def attention_block(g, qh):
                """Heads (2g, 2g+1), query half qh (1024 queries)."""
                qoff = qh * 1024
                vt = [avp.tile([P, 512], F32, tag="av", name=f"vt{i}") for i in range(2)]
                cs = csp.tile([P, 512], F32, tag="cs")
                for t in vt:
                    nc.vector.memset(t[:], 0.0)
                nc.vector.memset(cs[:], 0.0)

                for kt in range(NKT):
                    ets = {}
                    for hp, pb in ((0, 0), (1, 64)):  # head-in-pair, partition base
                        ps_s = psp.tile([P, 1024], F32, tag="ps")
                        for qb in range(2):
                            nc.tensor.matmul(
                                ps_s[:, qb * 512 : (qb + 1) * 512],
                                lhsT=KT[g][pb : pb + 64, kt * P : (kt + 1) * P],
                                rhs=QT[g][
                                    pb : pb + 64,
                                    qoff + qb * 512 : qoff + (qb + 1) * 512,
                                ],
                                start=True,
                                stop=True,
                            )
                        et = expp.tile([P, 1024], BF16, tag="expT")
                        nc.scalar.activation(
                            et[:], ps_s[:], mybir.ActivationFunctionType.Exp
                        )
                        ets[hp] = et
                    for hp, pb in ((0, 0), (1, 64)):
                        h = 2 * g + hp  # local head index
                        et = ets[hp]
                        for qb in range(2):
                            nc.tensor.matmul(
                                vt[qb][pb : pb + 64, :],
                                lhsT=V_st[kt][:, h, 0:DK],
                                rhs=et[:, qb * 512 : (qb + 1) * 512],
                                start=False,
                                stop=(kt == NKT - 1),
                                skip_group_check=True,
                                tile_position=(0, pb),
                            )
                            cp = 64 * hp + 32 * qb
                            nc.tensor.matmul(
                                cs[cp : cp + 1, :],
                                lhsT=ones_bf[:],
                                rhs=et[:, qb * 512 : (qb + 1) * 512],
                                start=False,
                                stop=(kt == NKT - 1),
                                skip_group_check=True,
                                tile_position=(0, cp),
                            )

                # normalize: broadcast Z via selector matmul, then divide.
                # (matmul rhs must be SBUF, so evacuate the colsums first)
                cs_sb = rcp.tile([P, 512], F32, tag="cs_sb")
                nc.vector.tensor_copy(out=cs_sb[:], in_=cs[:])
                for qb in range(2):
                    bc = bcp.tile([P, 512], F32, tag="bc")
                    nc.tensor.matmul(
                        bc[:],
                        lhsT=sel[:, qb * P : (qb + 1) * P],
                        rhs=cs_sb[:],
                        start=True,
                        stop=True,
                    )
                    rc = rcp.tile([P, 512], F32, tag="rc")
                    nc.vector.reciprocal(rc[:], bc[:])
                    for hp, pb in ((0, 0), (1, 64)):
                        nc.vector.tensor_mul(
                            out=attn[g][
                                pb : pb + 64, qoff + qb * 512 : qoff + (qb + 1) * 512
                            ],
                            in0=vt[qb][pb : pb + 64, :],
                            in1=rc[pb : pb + 64, :],
                        )

            for g in range(NG):
                for qh in range(2):
                    attention_block(g, qh)

            # ---------------- output projection
            for st in range(NKT):
                for ob in range(2):
                    ps = psp.tile([P, 512], F32, tag="ps")
                    for ct in range(NG):
                        nc.tensor.matmul(
                            ps[:],
                            lhsT=attn[ct][:, st * P : (st + 1) * P],
                            rhs=woT_sb[:, ct, ob * 512 : (ob + 1) * 512],
                            start=(ct == 0),
                            stop=(ct == NG - 1),
                        )
                    ot = outp.tile([P, 512], F32, tag="out")
                    nc.vector.tensor_copy(out=ot[:], in_=ps[:])
                    nc.sync.dma_start(
                        out[st * P : (st + 1) * P, ob * 512 : (ob + 1) * 512], ot[:]
                    )

    _split_sync_waits(nc)
    return nc


_NC = None


def _get_nc():
    global _NC
    if _NC is None:
        _NC = build_nc()
    return _NC


# ---------------------------------------------------------------- host side
def make_in_maps(x, wq, wk, wv, wo):
    x = np.asarray(x, dtype=np.float32)
    wq = np.asarray(wq, dtype=np.float32)
    wk = np.asarray(wk, dtype=np.float32)
    wv = np.asarray(wv, dtype=np.float32)
    wo = np.asarray(wo, dtype=np.float32)
    in_maps = []
    for c in range(N_CORES):
        b, hg = c // 2, c % 2
        sl = slice(hg * DL, (hg + 1) * DL)
        xTc = np.ascontiguousarray(x[b].T).astype(BF16_NP)
        wqTc = np.ascontiguousarray((wq[sl] / 8.0).T).astype(BF16_NP)
        wkTc = np.ascontiguousarray(wk[sl].T).astype(BF16_NP)
        wvTc = np.ascontiguousarray(wv[sl].T).astype(BF16_NP)
        woTc = np.ascontiguousarray(wo[:, sl].T).astype(BF16_NP)
        in_maps.append(
            {"xT": xTc, "wqT": wqTc, "wkT": wkTc, "wvT": wvTc, "woT": woTc}
        )
    return in_maps


def gather(results):
    out = np.zeros((4, S, DM), dtype=np.float32)
    for c in range(N_CORES):
        out[c // 2] += results[c]["out"]
    return out


def kernel(x, wq, wk, wv, wo):
    from concourse.bass_utils import run_bass_kernel_spmd

    nc = _get_nc()
    in_maps = make_in_maps(x, wq, wk, wv, wo)
    res = run_bass_kernel_spmd(nc, in_maps, CORE_IDS)
    return gather(res.results)


# revision 10
# speedup vs baseline: 1.1270x; 1.1270x over previous
"""Multi-head self-attention (B=4, S=2048, D=1024, H=16) on 8 trn2 NeuronCores.

Sharding: batch (4) x head-group (2 groups of 8 heads) -> 8 cores.
Each core computes, for its (batch b, head-group hg):
  Q'^T = (wq_l/8) @ x_b^T            [512, 2048]   (1/sqrt(dk) folded into wq)
  K^T  = wk_l @ x_b^T                [512, 2048]
  V    = x_b @ wv_l^T                [2048, 512]
  per head h (8 local, dk=64), in transposed layout (keys on partitions):
    scoresT[k, q] = K_h @ Q'_h^T     (no max-subtraction: scores ~ N(0,4), exp
                                      of |s|<~12 is safe in fp32/bf16)
    expT = exp(scoresT)              (ScalarE, PSUM->SBUF bf16)
    unnormT[c, q] = V_h^T @ expT     (PE, accumulated over key tiles)
    Z[q] = ones^T @ expT             (PE colsum, same accumulation)
    attnT = unnormT / Z              (broadcast Z via selector-matmul + DVE mul)
  out_partial = attnT^T @ wo_l^T     [2048, 1024]  (row-parallel wo)
Host sums the two partials per batch (the "all-reduce" of row-parallel wo).
"""

import sys
import types

import ml_dtypes
import numpy as np

import bass_rust
import concourse.bass as bass
import concourse.mybir as mybir
import concourse.tile as tile

# ---------------------------------------------------------------- constants
S = 2048          # sequence length
DM = 1024         # model dim
DL = 512          # local (per-core) head dims = 8 heads * 64
DK = 64           # head dim
P = 128
NKT = S // P      # 16 key tiles
NG = DL // P      # 4 head-pairs (c-tiles / dq-tiles)
KD = DM // P      # 8 contraction tiles for projections
NSC = S // 512    # 4 s-chunks for projections
F32 = mybir.dt.float32
BF16 = mybir.dt.bfloat16
BF16_NP = ml_dtypes.bfloat16

N_CORES = 8
CORE_IDS = list(range(N_CORES))


# ------------------------------------------------- walrus sync-wait workaround
def _split_sync_waits(nc, limit=1):
    """This toolchain's walrus codegen rejects instructions carrying more than
    one sync-wait command.  Move excess waits onto dedicated same-engine nops
    inserted immediately before the instruction (sequential waits on the same
    engine queue are semantically identical to multiple waits on one inst)."""
    fn = nc.m.functions[0]
    snapshots = [(bb, list(bb.instructions)) for bb in fn.blocks]
    plans = []
    for _bb, insts in snapshots:
        plan = {}
        for idx, inst in enumerate(insts):
            si = inst.sync_info
            waits = list(si.on_wait) if si and si.on_wait else []
            if len(waits) > limit:
                pre, keep = waits[:-limit], waits[-limit:]
                nops = []
                for w in pre:
                    ni = nc.engines[inst.engine].nop(nofuse=True, hint="wsplit").ins
                    ni.sync_info = bass_rust.SyncInfo(on_wait=[w], on_update=[])
                    nops.append(ni)
                si.on_wait = keep
                plan[idx] = nops
        plans.append(plan)
    # Rebuild every block from its pre-pass snapshot plus insertions; this also
    # drops the fresh nops from wherever bass appended them at creation time.
    for (bb, insts), plan in zip(snapshots, plans):
        out = []
        for idx, inst in enumerate(insts):
            out.extend(plan.get(idx, ()))
            out.append(inst)
        bb.instructions = out


# ---------------------------------------------------------------- the program
def build_nc():
    """Build the SPMD per-core Bass program (identical on all 8 cores)."""
    nc = bass.Bass()

    xT = nc.declare_dram_parameter("xT", [DM, S], BF16, isOutput=False)
    wqT = nc.declare_dram_parameter("wqT", [DM, DL], BF16, isOutput=False)
    wkT = nc.declare_dram_parameter("wkT", [DM, DL], BF16, isOutput=False)
    wvT = nc.declare_dram_parameter("wvT", [DM, DL], BF16, isOutput=False)
    woT = nc.declare_dram_parameter("woT", [DL, DM], BF16, isOutput=False)
    out = nc.declare_dram_parameter("out", [S, DM], F32, isOutput=True)

    with tile.TileContext(nc) as tc:
        with (
            tc.tile_pool(name="big", bufs=1) as big,
            tc.tile_pool(name="expT", bufs=6) as expp,
            tc.tile_pool(name="rc", bufs=2) as rcp,
            tc.tile_pool(name="outsb", bufs=3) as outp,
            tc.tile_pool(name="ps", bufs=2, space="PSUM") as psp,
            tc.tile_pool(name="av", bufs=2, space="PSUM") as avp,
            tc.tile_pool(name="cs", bufs=1, space="PSUM") as csp,
            tc.tile_pool(name="bc", bufs=1, space="PSUM") as bcp,
        ):
            # ---------------- load everything from DRAM
            xT_sb = big.tile([P, KD, S], BF16, tag="xT")
            nc.sync.dma_start(xT_sb[:], xT.rearrange("(kd p) s -> p kd s", p=P))
            w_sb = {}
            for name, dram in (("wq", wqT), ("wk", wkT), ("wv", wvT)):
                w_sb[name] = big.tile([P, KD, DL], BF16, tag=name, name=name)
                nc.sync.dma_start(
                    w_sb[name][:], dram.rearrange("(kd p) m -> p kd m", p=P)
                )
            woT_sb = big.tile([P, NG, DM], BF16, tag="wo")
            nc.sync.dma_start(woT_sb[:], woT.rearrange("(ct p) o -> p ct o", p=P))

            # ---------------- constants
            ones_bf = big.tile([P, 1], BF16, tag="ones")
            nc.vector.memset(ones_bf[:], 1.0)
            # selector for broadcasting Z (rows {0,32}->0:64, {64,96}->64:128)
            sel = big.tile([P, 2 * P], F32, tag="sel")
            nc.vector.memset(sel[:], 0.0)
            nc.vector.memset(sel[0:1, 0:64], 1.0)        # qb0, head A
            nc.vector.memset(sel[64:65, 64:128], 1.0)    # qb0, head B
            nc.vector.memset(sel[32:33, 128:192], 1.0)   # qb1, head A
            nc.vector.memset(sel[96:97, 192:256], 1.0)   # qb1, head B

            # persistent activation tensors
            QT = [big.tile([P, S], BF16, tag=f"QT{g}", name=f"QT{g}") for g in range(NG)]
            KT = [big.tile([P, S], BF16, tag=f"KT{g}", name=f"KT{g}") for g in range(NG)]
            V_st = [big.tile([P, 8, DK + 1], BF16, tag=f"V{st}", name=f"V{st}") for st in range(NKT)]
            for st in range(NKT):
                nc.vector.memset(V_st[st][:, :, DK], 1.0)  # spare ones column
            attn = [big.tile([P, S], BF16, tag=f"attn{g}", name=f"attn{g}") for g in range(NG)]

            # ---------------- projections
            def proj_qk(dst, w, g):
                """dst[:, :] (tile g) = rows [128g,128g+128) of W' @ x^T."""
                for sc in range(NSC):
                    ps = psp.tile([P, 512], F32, tag="ps")
                    for kd in range(KD):
                        nc.tensor.matmul(
                            ps[:],
                            lhsT=w[:, kd, g * P : (g + 1) * P],
                            rhs=xT_sb[:, kd, sc * 512 : (sc + 1) * 512],
                            start=(kd == 0),
                            stop=(kd == KD - 1),
                        )
                    nc.vector.tensor_copy(
                        out=dst[:, sc * 512 : (sc + 1) * 512], in_=ps[:]
                    )

            def proj_v(st):
                ps = psp.tile([P, 512], F32, tag="ps")
                for kd in range(KD):
                    nc.tensor.matmul(
                        ps[:],
                        lhsT=xT_sb[:, kd, st * P : (st + 1) * P],
                        rhs=w_sb["wv"][:, kd, :],
                        start=(kd == 0),
                        stop=(kd == KD - 1),
                    )
                nc.vector.tensor_copy(
                    out=V_st[st][:, :, 0:DK],
                    in_=ps.rearrange("p (h c) -> p h c", c=DK),
                )

            # g=0 first so attention can start early
            proj_qk(KT[0], w_sb["wk"], 0)
            proj_qk(QT[0], w_sb["wq"], 0)
            for st in range(NKT):
                proj_v(st)
            for g in range(1, NG):
                proj_qk(KT[g], w_sb["wk"], g)
                proj_qk(QT[g], w_sb["wq"], g)

            # ---------------- attention
            def attention_block(g, qh):
                """Heads (2g, 2g+1), query half qh (1024 queries)."""
                qoff = qh * 1024
                vt = [avp.tile([P, 512], F32, tag="av", name=f"vt{i}") for i in range(2)]
                cs = csp.tile([P, 512], F32, tag="cs")
                for t in vt:
                    nc.vector.memset(t[:], 0.0)
                nc.vector.memset(cs[:], 0.0)

                # Emission order matters: matmuls placed adjacently whose
                # array tile-positions are disjoint run concurrently on the PE
                # (scores: row groups 0/64; V: col groups 0-1/2-3; colsums:
                # col groups at 32-strips 0/32/64/96).
                for kt in range(NKT):
                    ps_s = {}
                    for qb in range(2):
                        for hp, pb in ((0, 0), (1, 64)):
                            if qb == 0:
                                ps_s[hp] = psp.tile(
                                    [P, 1024], F32, tag="ps", name=f"ps_s{hp}"
                                )
                            nc.tensor.matmul(
                                ps_s[hp][:, qb * 512 : (qb + 1) * 512],
                                lhsT=KT[g][pb : pb + 64, kt * P : (kt + 1) * P],
                                rhs=QT[g][
                                    pb : pb + 64,
                                    qoff + qb * 512 : qoff + (qb + 1) * 512,
                                ],
                                start=True,
                                stop=True,
                            )
                    ets = {}
                    for hp in (0, 1):
                        et = expp.tile([P, 1024], BF16, tag="expT", name=f"et{hp}")
                        nc.scalar.activation(
                            et[:], ps_s[hp][:], mybir.ActivationFunctionType.Exp
                        )
                        ets[hp] = et
                    for qb in range(2):
                        for hp, pb in ((0, 0), (1, 64)):
                            nc.tensor.matmul(
                                vt[qb][pb : pb + 64, :],
                                lhsT=V_st[kt][:, 2 * g + hp, 0:DK],
                                rhs=ets[hp][:, qb * 512 : (qb + 1) * 512],
                                start=False,
                                stop=(kt == NKT - 1),
                                skip_group_check=True,
                                tile_position=(0, pb),
                            )
                    for hp in (0, 1):
                        for qb in range(2):
                            cp = 64 * hp + 32 * qb
                            nc.tensor.matmul(
                                cs[cp : cp + 1, :],
                                lhsT=ones_bf[:],
                                rhs=ets[hp][:, qb * 512 : (qb + 1) * 512],
                                start=False,
                                stop=(kt == NKT - 1),
                                skip_group_check=True,
                                tile_position=(0, cp),
                            )

                # normalize: broadcast Z via selector matmul, then divide.
                # (matmul rhs must be SBUF, so evacuate the colsums first)
                cs_sb = rcp.tile([P, 512], F32, tag="cs_sb")
                nc.vector.tensor_copy(out=cs_sb[:], in_=cs[:])
                for qb in range(2):
                    bc = bcp.tile([P, 512], F32, tag="bc")
                    nc.tensor.matmul(
                        bc[:],
                        lhsT=sel[:, qb * P : (qb + 1) * P],
                        rhs=cs_sb[:],
                        start=True,
                        stop=True,
                    )
                    rc = rcp.tile([P, 512], F32, tag="rc")
                    nc.vector.reciprocal(rc[:], bc[:])
                    for hp, pb in ((0, 0), (1, 64)):
                        nc.vector.tensor_mul(
                            out=attn[g][
                                pb : pb + 64, qoff + qb * 512 : qoff + (qb + 1) * 512
                            ],
                            in0=vt[qb][pb : pb + 64, :],
                            in1=rc[pb : pb + 64, :],
                        )

            for g in range(NG):
                for qh in range(2):
                    attention_block(g, qh)

            # ---------------- output projection
            for st in range(NKT):
                for ob in range(2):
                    ps = psp.tile([P, 512], F32, tag="ps")
                    for ct in range(NG):
                        nc.tensor.matmul(
                            ps[:],
                            lhsT=attn[ct][:, st * P : (st + 1) * P],
                            rhs=woT_sb[:, ct, ob * 512 : (ob + 1) * 512],
                            start=(ct == 0),
                            stop=(ct == NG - 1),
                        )
                    ot = outp.tile([P, 512], F32, tag="out")
                    nc.vector.tensor_copy(out=ot[:], in_=ps[:])
                    nc.sync.dma_start(
                        out[st * P : (st + 1) * P, ob * 512 : (ob + 1) * 512], ot[:]
                    )

    _split_sync_waits(nc)
    return nc


_NC = None


def _get_nc():
    global _NC
    if _NC is None:
        _NC = build_nc()
    return _NC


# ---------------------------------------------------------------- host side
def make_in_maps(x, wq, wk, wv, wo):
    x = np.asarray(x, dtype=np.float32)
    wq = np.asarray(wq, dtype=np.float32)
    wk = np.asarray(wk, dtype=np.float32)
    wv = np.asarray(wv, dtype=np.float32)
    wo = np.asarray(wo, dtype=np.float32)
    in_maps = []
    for c in range(N_CORES):
        b, hg = c // 2, c % 2
        sl = slice(hg * DL, (hg + 1) * DL)
        xTc = np.ascontiguousarray(x[b].T).astype(BF16_NP)
        wqTc = np.ascontiguousarray((wq[sl] / 8.0).T).astype(BF16_NP)
        wkTc = np.ascontiguousarray(wk[sl].T).astype(BF16_NP)
        wvTc = np.ascontiguousarray(wv[sl].T).astype(BF16_NP)
        woTc = np.ascontiguousarray(wo[:, sl].T).astype(BF16_NP)
        in_maps.append(
            {"xT": xTc, "wqT": wqTc, "wkT": wkTc, "wvT": wvTc, "woT": woTc}
        )
    return in_maps


def gather(results):
    out = np.zeros((4, S, DM), dtype=np.float32)
    for c in range(N_CORES):
        out[c // 2] += results[c]["out"]
    return out


def kernel(x, wq, wk, wv, wo):
    from concourse.bass_utils import run_bass_kernel_spmd

    nc = _get_nc()
    in_maps = make_in_maps(x, wq, wk, wv, wo)
    res = run_bass_kernel_spmd(nc, in_maps, CORE_IDS)
    return gather(res.results)


# revision 13
# speedup vs baseline: 1.2429x; 1.1028x over previous
"""Multi-head self-attention (B=4, S=2048, D=1024, H=16) on 8 trn2 NeuronCores.

Sharding: batch (4) x head-group (2 groups of 8 heads) -> 8 cores.
Each core computes, for its (batch b, head-group hg):
  Q'^T = (wq_l/8) @ x_b^T            [512, 2048]   (1/sqrt(dk) folded into wq)
  K^T  = wk_l @ x_b^T                [512, 2048]
  V    = x_b @ wv_l^T                [2048, 512]
  per head h (8 local, dk=64), in transposed layout (keys on partitions):
    scoresT[k, q] = K_h @ Q'_h^T     (no max-subtraction: scores ~ N(0,4), exp
                                      of |s|<~12 is safe in fp32/bf16)
    expT = exp(scoresT)              (ScalarE, PSUM->SBUF bf16)
    unnormT[c, q] = V_h^T @ expT     (PE, accumulated over key tiles)
    Z[q] = ones^T @ expT             (PE colsum, same accumulation)
    attnT = unnormT / Z              (broadcast Z via selector-matmul + DVE mul)
  out_partial = attnT^T @ wo_l^T     [2048, 1024]  (row-parallel wo)
Host sums the two partials per batch (the "all-reduce" of row-parallel wo).
"""

import sys
import types

import ml_dtypes
import numpy as np

import bass_rust
import concourse.bass as bass
import concourse.mybir as mybir
import concourse.tile as tile

# ---------------------------------------------------------------- constants
S = 2048          # sequence length
DM = 1024         # model dim
DL = 512          # local (per-core) head dims = 8 heads * 64
DK = 64           # head dim
P = 128
NKT = S // P      # 16 key tiles
NG = DL // P      # 4 head-pairs (c-tiles / dq-tiles)
KD = DM // P      # 8 contraction tiles for projections
NSC = S // 512    # 4 s-chunks for projections
F32 = mybir.dt.float32
BF16 = mybir.dt.bfloat16
BF16_NP = ml_dtypes.bfloat16

N_CORES = 8
CORE_IDS = list(range(N_CORES))


# ------------------------------------------------- walrus sync-wait workaround
def _split_sync_waits(nc, limit=1):
    """This toolchain's walrus codegen rejects instructions carrying more than
    one sync-wait command.  Move excess waits onto dedicated same-engine nops
    inserted immediately before the instruction (sequential waits on the same
    engine queue are semantically identical to multiple waits on one inst)."""
    fn = nc.m.functions[0]
    snapshots = [(bb, list(bb.instructions)) for bb in fn.blocks]
    plans = []
    for _bb, insts in snapshots:
        plan = {}
        for idx, inst in enumerate(insts):
            si = inst.sync_info
            waits = list(si.on_wait) if si and si.on_wait else []
            if len(waits) > limit:
                pre, keep = waits[:-limit], waits[-limit:]
                nops = []
                for w in pre:
                    ni = nc.engines[inst.engine].nop(nofuse=True, hint="wsplit").ins
                    ni.sync_info = bass_rust.SyncInfo(on_wait=[w], on_update=[])
                    nops.append(ni)
                si.on_wait = keep
                plan[idx] = nops
        plans.append(plan)
    # Rebuild every block from its pre-pass snapshot plus insertions; this also
    # drops the fresh nops from wherever bass appended them at creation time.
    for (bb, insts), plan in zip(snapshots, plans):
        out = []
        for idx, inst in enumerate(insts):
            out.extend(plan.get(idx, ()))
            out.append(inst)
        bb.instructions = out


# ---------------------------------------------------------------- the program
def build_nc():
    """Build the SPMD per-core Bass program (identical on all 8 cores)."""
    nc = bass.Bass()

    xT = nc.declare_dram_parameter("xT", [DM, S], BF16, isOutput=False)
    wqT = nc.declare_dram_parameter("wqT", [DM, DL], BF16, isOutput=False)
    wkT = nc.declare_dram_parameter("wkT", [DM, DL], BF16, isOutput=False)
    wvT = nc.declare_dram_parameter("wvT", [DM, DL], BF16, isOutput=False)
    woT = nc.declare_dram_parameter("woT", [DL, DM], BF16, isOutput=False)
    out = nc.declare_dram_parameter("out", [S, DM], F32, isOutput=True)

    with tile.TileContext(nc) as tc:
        with (
            tc.tile_pool(name="big", bufs=1) as big,
            tc.tile_pool(name="expT", bufs=6) as expp,
            tc.tile_pool(name="rc", bufs=2) as rcp,
            tc.tile_pool(name="outsb", bufs=3) as outp,
            tc.tile_pool(name="dram", bufs=2, space="DRAM") as dramp,
            tc.tile_pool(name="ps", bufs=2, space="PSUM") as psp,
            tc.tile_pool(name="av", bufs=3, space="PSUM") as avp,
            tc.tile_pool(name="cs", bufs=1, space="PSUM") as csp,
        ):
            # ---------------- load everything from DRAM
            xT_sb = big.tile([P, KD, S], BF16, tag="xT")
            nc.sync.dma_start(xT_sb[:], xT.rearrange("(kd p) s -> p kd s", p=P))
            w_sb = {}
            for name, dram in (("wq", wqT), ("wk", wkT), ("wv", wvT)):
                w_sb[name] = big.tile([P, KD, DL], BF16, tag=name, name=name)
                nc.sync.dma_start(
                    w_sb[name][:], dram.rearrange("(kd p) m -> p kd m", p=P)
                )
            woT_sb = big.tile([P, NG, DM], BF16, tag="wo")
            nc.sync.dma_start(woT_sb[:], woT.rearrange("(ct p) o -> p ct o", p=P))

            # ---------------- constants
            ones_bf = big.tile([P, 1], BF16, tag="ones")
            nc.vector.memset(ones_bf[:], 1.0)

            # persistent activation tensors
            QT = [big.tile([P, S], BF16, tag=f"QT{g}", name=f"QT{g}") for g in range(NG)]
            KT = [big.tile([P, S], BF16, tag=f"KT{g}", name=f"KT{g}") for g in range(NG)]
            V_st = [big.tile([P, 8, DK + 1], BF16, tag=f"V{st}", name=f"V{st}") for st in range(NKT)]
            for st in range(NKT):
                nc.vector.memset(V_st[st][:, :, DK], 1.0)  # spare ones column
            attn = [big.tile([P, S], BF16, tag=f"attn{g}", name=f"attn{g}") for g in range(NG)]

            # ---------------- projections
            def proj_qk(dst, w, g):
                """dst[:, :] (tile g) = rows [128g,128g+128) of W' @ x^T."""
                for sc in range(NSC):
                    ps = psp.tile([P, 512], F32, tag="ps")
                    for kd in range(KD):
                        nc.tensor.matmul(
                            ps[:],
                            lhsT=w[:, kd, g * P : (g + 1) * P],
                            rhs=xT_sb[:, kd, sc * 512 : (sc + 1) * 512],
                            start=(kd == 0),
                            stop=(kd == KD - 1),
                        )
                    nc.vector.tensor_copy(
                        out=dst[:, sc * 512 : (sc + 1) * 512], in_=ps[:]
                    )

            def proj_v(st):
                ps = psp.tile([P, 512], F32, tag="ps")
                for kd in range(KD):
                    nc.tensor.matmul(
                        ps[:],
                        lhsT=xT_sb[:, kd, st * P : (st + 1) * P],
                        rhs=w_sb["wv"][:, kd, :],
                        start=(kd == 0),
                        stop=(kd == KD - 1),
                    )
                nc.vector.tensor_copy(
                    out=V_st[st][:, :, 0:DK],
                    in_=ps.rearrange("p (h c) -> p h c", c=DK),
                )

            # g=0 first so attention can start early
            proj_qk(KT[0], w_sb["wk"], 0)
            proj_qk(QT[0], w_sb["wq"], 0)
            for st in range(NKT):
                proj_v(st)
            for g in range(1, NG):
                proj_qk(KT[g], w_sb["wk"], g)
                proj_qk(QT[g], w_sb["wq"], g)

            # ---------------- attention
            def attention_block(g, qh):
                """Heads (2g, 2g+1), query half qh (1024 queries)."""
                qoff = qh * 1024
                vt = [avp.tile([P, 512], F32, tag="av", name=f"vt{i}") for i in range(2)]
                cs = csp.tile([P, 512], F32, tag="cs")
                for t in vt:
                    nc.vector.memset(t[:], 0.0)
                nc.vector.memset(cs[:], 0.0)

                # Emission order matters: matmuls placed adjacently whose
                # array tile-positions are disjoint run concurrently on the PE
                # (scores: row groups 0/64; V: col groups 0-1/2-3; colsums:
                # col groups at 32-strips 0/32/64/96).
                for kt in range(NKT):
                    ps_s = {}
                    for qb in range(2):
                        for hp, pb in ((0, 0), (1, 64)):
                            if qb == 0:
                                ps_s[hp] = psp.tile(
                                    [P, 1024], F32, tag="ps", name=f"ps_s{hp}"
                                )
                            nc.tensor.matmul(
                                ps_s[hp][:, qb * 512 : (qb + 1) * 512],
                                lhsT=KT[g][pb : pb + 64, kt * P : (kt + 1) * P],
                                rhs=QT[g][
                                    pb : pb + 64,
                                    qoff + qb * 512 : qoff + (qb + 1) * 512,
                                ],
                                start=True,
                                stop=True,
                            )
                    ets = {}
                    for hp in (0, 1):
                        et = expp.tile([P, 1024], BF16, tag="expT", name=f"et{hp}")
                        nc.scalar.activation(
                            et[:], ps_s[hp][:], mybir.ActivationFunctionType.Exp
                        )
                        ets[hp] = et
                    for qb in range(2):
                        for hp, pb in ((0, 0), (1, 64)):
                            nc.tensor.matmul(
                                vt[qb][pb : pb + 64, :],
                                lhsT=V_st[kt][:, 2 * g + hp, 0:DK],
                                rhs=ets[hp][:, qb * 512 : (qb + 1) * 512],
                                start=False,
                                stop=(kt == NKT - 1),
                                skip_group_check=True,
                                tile_position=(0, pb),
                            )
                    for hp in (0, 1):
                        for qb in range(2):
                            cp = 64 * hp + 32 * qb
                            nc.tensor.matmul(
                                cs[cp : cp + 1, :],
                                lhsT=ones_bf[:],
                                rhs=ets[hp][:, qb * 512 : (qb + 1) * 512],
                                start=False,
                                stop=(kt == NKT - 1),
                                skip_group_check=True,
                                tile_position=(0, cp),
                            )

                # Normalization, fully off the PE/ACT critical path:
                # evacuate colsums + unnormalized attn from PSUM (frees banks),
                # reciprocal once (junk rows -> inf, never read), broadcast the
                # four Z-reciprocal rows across partitions via a DRAM
                # round-trip (DMA src with partition-step 0), then DVE muls.
                cs_sb = rcp.tile([P, 512], F32, tag="cs_sb")
                nc.vector.tensor_copy(out=cs_sb[:], in_=cs[:])
                cs_rc = rcp.tile([P, 512], F32, tag="cs_rc")
                nc.vector.reciprocal(cs_rc[:], cs_sb[:])
                un = [
                    rcp.tile([P, 512], F32, tag="unnorm", name=f"un{qb}")
                    for qb in range(2)
                ]
                for qb in range(2):
                    nc.vector.tensor_copy(out=un[qb][:], in_=vt[qb][:])
                zd = dramp.tile([4, 512], F32, name="zd")
                # zd rows: 0=(A,qb0) 1=(A,qb1) 2=(B,qb0) 3=(B,qb1)
                nc.sync.dma_start(zd[:], cs_rc[0:128:32, :])
                for qb in range(2):
                    rcb = rcp.tile([P, 512], F32, tag="rcb", name=f"rcb{qb}")
                    nc.sync.dma_start(
                        rcb[0:64, :], zd[qb, None, :].to_broadcast([64, 512])
                    )
                    nc.sync.dma_start(
                        rcb[64:128, :], zd[qb + 2, None, :].to_broadcast([64, 512])
                    )
                    for hp, pb in ((0, 0), (1, 64)):
                        nc.vector.tensor_mul(
                            out=attn[g][
                                pb : pb + 64, qoff + qb * 512 : qoff + (qb + 1) * 512
                            ],
                            in0=un[qb][pb : pb + 64, :],
                            in1=rcb[pb : pb + 64, :],
                        )

            for g in range(NG):
                for qh in range(2):
                    attention_block(g, qh)

            # ---------------- output projection
            for st in range(NKT):
                for ob in range(2):
                    ps = psp.tile([P, 512], F32, tag="ps")
                    for ct in range(NG):
                        nc.tensor.matmul(
                            ps[:],
                            lhsT=attn[ct][:, st * P : (st + 1) * P],
                            rhs=woT_sb[:, ct, ob * 512 : (ob + 1) * 512],
                            start=(ct == 0),
                            stop=(ct == NG - 1),
                        )
                    ot = outp.tile([P, 512], F32, tag="out")
                    nc.vector.tensor_copy(out=ot[:], in_=ps[:])
                    nc.sync.dma_start(
                        out[st * P : (st + 1) * P, ob * 512 : (ob + 1) * 512], ot[:]
                    )

    _split_sync_waits(nc)
    return nc


_NC = None


def _get_nc():
    global _NC
    if _NC is None:
        _NC = build_nc()
    return _NC


# ---------------------------------------------------------------- host side
def make_in_maps(x, wq, wk, wv, wo):
    x = np.asarray(x, dtype=np.float32)
    wq = np.asarray(wq, dtype=np.float32)
    wk = np.asarray(wk, dtype=np.float32)
    wv = np.asarray(wv, dtype=np.float32)
    wo = np.asarray(wo, dtype=np.float32)
    in_maps = []
    for c in range(N_CORES):
        b, hg = c // 2, c % 2
        sl = slice(hg * DL, (hg + 1) * DL)
        xTc = np.ascontiguousarray(x[b].T).astype(BF16_NP)
        wqTc = np.ascontiguousarray((wq[sl] / 8.0).T).astype(BF16_NP)
        wkTc = np.ascontiguousarray(wk[sl].T).astype(BF16_NP)
        wvTc = np.ascontiguousarray(wv[sl].T).astype(BF16_NP)
        woTc = np.ascontiguousarray(wo[:, sl].T).astype(BF16_NP)
        in_maps.append(
            {"xT": xTc, "wqT": wqTc, "wkT": wkTc, "wvT": wvTc, "woT": woTc}
        )
    return in_maps


def gather(results):
    out = np.zeros((4, S, DM), dtype=np.float32)
    for c in range(N_CORES):
        out[c // 2] += results[c]["out"]
    return out


def kernel(x, wq, wk, wv, wo):
    from concourse.bass_utils import run_bass_kernel_spmd

    nc = _get_nc()
    in_maps = make_in_maps(x, wq, wk, wv, wo)
    res = run_bass_kernel_spmd(nc, in_maps, CORE_IDS)
    return gather(res.results)


# revision 15
# speedup vs baseline: 1.7289x; 1.3911x over previous
"""Multi-head self-attention (B=4, S=2048, D=1024, H=16) on 8 trn2 NeuronCores.

Sharding: batch (4) x head-group (2 groups of 8 heads) -> 8 cores.
Each core computes, for its (batch b, head-group hg):
  Q'^T = (wq_l/8) @ x_b^T            [512, 2048]   (1/sqrt(dk) folded into wq)
  K^T  = wk_l @ x_b^T                [512, 2048]
  V    = x_b @ wv_l^T                [2048, 512]
  per head h (8 local, dk=64), in transposed layout (keys on partitions):
    scoresT[k, q] = K_h @ Q'_h^T     (no max-subtraction: scores ~ N(0,4), exp
                                      of |s|<~12 is safe in fp32/bf16)
    expT = exp(scoresT)              (ScalarE, PSUM->SBUF bf16)
    unnormT[c, q] = V_h^T @ expT     (PE, accumulated over key tiles)
    Z[q] = ones^T @ expT             (PE colsum, same accumulation)
    attnT = unnormT / Z              (broadcast Z via selector-matmul + DVE mul)
  out_partial = attnT^T @ wo_l^T     [2048, 1024]  (row-parallel wo)
Host sums the two partials per batch (the "all-reduce" of row-parallel wo).
"""

import sys
import types

import ml_dtypes
import numpy as np

import bass_rust
import concourse.bass as bass
import concourse.mybir as mybir
import concourse.tile as tile

# ---------------------------------------------------------------- constants
S = 2048          # sequence length
DM = 1024         # model dim
DL = 512          # local (per-core) head dims = 8 heads * 64
DK = 64           # head dim
P = 128
NKT = S // P      # 16 key tiles
NG = DL // P      # 4 head-pairs (c-tiles / dq-tiles)
KD = DM // P      # 8 contraction tiles for projections
NSC = S // 512    # 4 s-chunks for projections
F32 = mybir.dt.float32
BF16 = mybir.dt.bfloat16
BF16_NP = ml_dtypes.bfloat16

N_CORES = 8
CORE_IDS = list(range(N_CORES))


# ------------------------------------------------- walrus sync-wait workaround
def _split_sync_waits(nc, limit=1):
    """This toolchain's walrus codegen rejects instructions carrying more than
    one sync-wait command.  Move excess waits onto dedicated same-engine nops
    inserted immediately before the instruction (sequential waits on the same
    engine queue are semantically identical to multiple waits on one inst)."""
    fn = nc.m.functions[0]
    snapshots = [(bb, list(bb.instructions)) for bb in fn.blocks]
    plans = []
    for _bb, insts in snapshots:
        plan = {}
        for idx, inst in enumerate(insts):
            si = inst.sync_info
            waits = list(si.on_wait) if si and si.on_wait else []
            if len(waits) > limit:
                pre, keep = waits[:-limit], waits[-limit:]
                nops = []
                for w in pre:
                    ni = nc.engines[inst.engine].nop(nofuse=True, hint="wsplit").ins
                    ni.sync_info = bass_rust.SyncInfo(on_wait=[w], on_update=[])
                    nops.append(ni)
                si.on_wait = keep
                plan[idx] = nops
        plans.append(plan)
    # Rebuild every block from its pre-pass snapshot plus insertions; this also
    # drops the fresh nops from wherever bass appended them at creation time.
    for (bb, insts), plan in zip(snapshots, plans):
        out = []
        for idx, inst in enumerate(insts):
            out.extend(plan.get(idx, ()))
            out.append(inst)
        bb.instructions = out


# ---------------------------------------------------------------- the program
def build_nc():
    """Build the SPMD per-core Bass program (identical on all 8 cores)."""
    nc = bass.Bass()

    xT = nc.declare_dram_parameter("xT", [DM, S], BF16, isOutput=False)
    wqT = nc.declare_dram_parameter("wqT", [DM, DL], BF16, isOutput=False)
    wkT = nc.declare_dram_parameter("wkT", [DM, DL], BF16, isOutput=False)
    wvT = nc.declare_dram_parameter("wvT", [DM, DL], BF16, isOutput=False)
    woT = nc.declare_dram_parameter("woT", [DL, DM], BF16, isOutput=False)
    out = nc.declare_dram_parameter("out", [S, DM], F32, isOutput=True)

    with tile.TileContext(nc) as tc:
        with (
            tc.tile_pool(name="big", bufs=1) as big,
            tc.tile_pool(name="expT", bufs=6) as expp,
            tc.tile_pool(name="rc", bufs=2) as rcp,
            tc.tile_pool(name="outsb", bufs=3) as outp,
            tc.tile_pool(name="dram", bufs=2, space="DRAM") as dramp,
            tc.tile_pool(name="ps", bufs=2, space="PSUM") as psp,
            tc.tile_pool(name="av", bufs=4, space="PSUM") as avp,
        ):
            # ---------------- load everything from DRAM
            xT_sb = big.tile([P, KD, S], BF16, tag="xT")
            nc.sync.dma_start(xT_sb[:], xT.rearrange("(kd p) s -> p kd s", p=P))
            w_sb = {}
            for name, dram in (("wq", wqT), ("wk", wkT), ("wv", wvT)):
                w_sb[name] = big.tile([P, KD, DL], BF16, tag=name, name=name)
                nc.sync.dma_start(
                    w_sb[name][:], dram.rearrange("(kd p) m -> p kd m", p=P)
                )
            woT_sb = big.tile([P, NG, DM], BF16, tag="wo")
            nc.sync.dma_start(woT_sb[:], woT.rearrange("(ct p) o -> p ct o", p=P))

            # ---------------- constants

            # persistent activation tensors
            QT = [big.tile([P, S], BF16, tag=f"QT{g}", name=f"QT{g}") for g in range(NG)]
            KT = [big.tile([P, S], BF16, tag=f"KT{g}", name=f"KT{g}") for g in range(NG)]
            V_st = [big.tile([P, 8, DK + 1], BF16, tag=f"V{st}", name=f"V{st}") for st in range(NKT)]
            for st in range(NKT):
                nc.vector.memset(V_st[st][:, :, DK], 1.0)  # spare ones column
            attn = [big.tile([P, S], BF16, tag=f"attn{g}", name=f"attn{g}") for g in range(NG)]

            # ---------------- projections
            def proj_qk(dst, w, g):
                """dst[:, :] (tile g) = rows [128g,128g+128) of W' @ x^T."""
                for sc in range(NSC):
                    ps = psp.tile([P, 512], F32, tag="ps")
                    for kd in range(KD):
                        nc.tensor.matmul(
                            ps[:],
                            lhsT=w[:, kd, g * P : (g + 1) * P],
                            rhs=xT_sb[:, kd, sc * 512 : (sc + 1) * 512],
                            start=(kd == 0),
                            stop=(kd == KD - 1),
                        )
                    nc.vector.tensor_copy(
                        out=dst[:, sc * 512 : (sc + 1) * 512], in_=ps[:]
                    )

            def proj_v(st):
                ps = psp.tile([P, 512], F32, tag="ps")
                for kd in range(KD):
                    nc.tensor.matmul(
                        ps[:],
                        lhsT=xT_sb[:, kd, st * P : (st + 1) * P],
                        rhs=w_sb["wv"][:, kd, :],
                        start=(kd == 0),
                        stop=(kd == KD - 1),
                    )
                nc.vector.tensor_copy(
                    out=V_st[st][:, :, 0:DK],
                    in_=ps.rearrange("p (h c) -> p h c", c=DK),
                )

            # g=0 first so attention can start early
            proj_qk(KT[0], w_sb["wk"], 0)
            proj_qk(QT[0], w_sb["wq"], 0)
            for st in range(NKT):
                proj_v(st)
            for g in range(1, NG):
                proj_qk(KT[g], w_sb["wk"], g)
                proj_qk(QT[g], w_sb["wq"], g)

            # ---------------- attention
            def attention_block(g, qh):
                """Heads (2g, 2g+1), query half qh (1024 queries).

                scoresT/exp are ACT-paced; V matmuls (with the ones-column of
                V_aug producing the softmax denominator in PSUM row 64) are
                software-pipelined one kt behind so the PE never head-of-line
                blocks the scores chain.  Normalization runs entirely off the
                critical path: DVE evictions, one reciprocal, DMA partition
                broadcast via DRAM, DVE muls, and an SBUF->SBUF DMA partition
                shift for head B's rows.
                """
                qoff = qh * 1024
                vt = [
                    avp.tile([65, 512], F32, tag="av", name=f"vt{i}")
                    for i in range(4)  # (A,qb0) (A,qb1) (B,qb0) (B,qb1)
                ]

                ets = {}

                def emit_scores_exp(kt):
                    for hp, pb in ((0, 0), (1, 64)):
                        ps_s = psp.tile([P, 1024], F32, tag="ps", name=f"ps_s{hp}")
                        for qb in range(2):
                            nc.tensor.matmul(
                                ps_s[:, qb * 512 : (qb + 1) * 512],
                                lhsT=KT[g][pb : pb + 64, kt * P : (kt + 1) * P],
                                rhs=QT[g][
                                    pb : pb + 64,
                                    qoff + qb * 512 : qoff + (qb + 1) * 512,
                                ],
                                start=True,
                                stop=True,
                            )
                        et = expp.tile([P, 1024], BF16, tag="expT", name=f"et{hp}")
                        nc.scalar.activation(
                            et[:], ps_s[:], mybir.ActivationFunctionType.Exp
                        )
                        ets[(kt, hp)] = et

                def emit_v(kt):
                    for hp in (0, 1):
                        et = ets.pop((kt, hp))
                        for qb in range(2):
                            nc.tensor.matmul(
                                vt[2 * hp + qb][:],
                                lhsT=V_st[kt][:, 2 * g + hp, 0 : DK + 1],
                                rhs=et[:, qb * 512 : (qb + 1) * 512],
                                start=(kt == 0),
                                stop=(kt == NKT - 1),
                            )

                for kt in range(NKT):
                    emit_scores_exp(kt)
                    if kt >= 1:
                        emit_v(kt - 1)
                emit_v(NKT - 1)

                # ---- normalization (off critical path) ----
                un = [
                    rcp.tile([65, 512], F32, tag=f"un{i}", name=f"un{i}")
                    for i in range(4)
                ]
                for i in range(4):
                    nc.vector.tensor_copy(out=un[i][:], in_=vt[i][:])
                # gather the 4 Z rows (partition 64 of each un tile) into [4,512]
                zsq = rcp.tile([4, 512], F32, tag="zsq")
                for i in range(4):
                    nc.sync.dma_start(zsq[i : i + 1, :], un[i][64:65, :])
                zrc = rcp.tile([4, 512], F32, tag="zrc")
                nc.vector.reciprocal(zrc[:], zsq[:])
                zd = dramp.tile([4, 512], F32, name="zd")
                nc.sync.dma_start(zd[:], zrc[:])
                # broadcast each Z-reciprocal row across 64 partitions
                rcb = [
                    rcp.tile([64, 512], F32, tag=f"rcb{i}", name=f"rcb{i}")
                    for i in range(4)
                ]
                for i in range(4):
                    nc.sync.dma_start(
                        rcb[i][:], zd[i, None, :].to_broadcast([64, 512])
                    )
                for qb in range(2):
                    # head A rows land directly in attn[g][0:64]
                    nc.vector.tensor_mul(
                        out=attn[g][0:64, qoff + qb * 512 : qoff + (qb + 1) * 512],
                        in0=un[qb][0:64, :],
                        in1=rcb[qb][:],
                    )
                    # head B: mul lane-aligned at partitions 0-63, then DMA
                    # partition-shift into attn[g][64:128]
                    bst = rcp.tile([64, 512], BF16, tag="bst", name=f"bst{qb}")
                    nc.vector.tensor_mul(
                        out=bst[:], in0=un[2 + qb][0:64, :], in1=rcb[2 + qb][:]
                    )
                    nc.sync.dma_start(
                        attn[g][64:128, qoff + qb * 512 : qoff + (qb + 1) * 512],
                        bst[:],
                    )

            for g in range(NG):
                for qh in range(2):
                    attention_block(g, qh)

            # ---------------- output projection
            for st in range(NKT):
                for ob in range(2):
                    ps = psp.tile([P, 512], F32, tag="ps")
                    for ct in range(NG):
                        nc.tensor.matmul(
                            ps[:],
                            lhsT=attn[ct][:, st * P : (st + 1) * P],
                            rhs=woT_sb[:, ct, ob * 512 : (ob + 1) * 512],
                            start=(ct == 0),
                            stop=(ct == NG - 1),
                        )
                    ot = outp.tile([P, 512], F32, tag="out")
                    nc.vector.tensor_copy(out=ot[:], in_=ps[:])
                    nc.sync.dma_start(
                        out[st * P : (st + 1) * P, ob * 512 : (ob + 1) * 512], ot[:]
                    )

    _split_sync_waits(nc)
    return nc


_NC = None


def _get_nc():
    global _NC
    if _NC is None:
        _NC = build_nc()
    return _NC


# ---------------------------------------------------------------- host side
def make_in_maps(x, wq, wk, wv, wo):
    x = np.asarray(x, dtype=np.float32)
    wq = np.asarray(wq, dtype=np.float32)
    wk = np.asarray(wk, dtype=np.float32)
    wv = np.asarray(wv, dtype=np.float32)
    wo = np.asarray(wo, dtype=np.float32)
    in_maps = []
    for c in range(N_CORES):
        b, hg = c // 2, c % 2
        sl = slice(hg * DL, (hg + 1) * DL)
        xTc = np.ascontiguousarray(x[b].T).astype(BF16_NP)
        wqTc = np.ascontiguousarray((wq[sl] / 8.0).T).astype(BF16_NP)
        wkTc = np.ascontiguousarray(wk[sl].T).astype(BF16_NP)
        wvTc = np.ascontiguousarray(wv[sl].T).astype(BF16_NP)
        woTc = np.ascontiguousarray(wo[:, sl].T).astype(BF16_NP)
        in_maps.append(
            {"xT": xTc, "wqT": wqTc, "wkT": wkTc, "wvT": wvTc, "woT": woTc}
        )
    return in_maps


def gather(results):
    out = np.zeros((4, S, DM), dtype=np.float32)
    for c in range(N_CORES):
        out[c // 2] += results[c]["out"]
    return out


def kernel(x, wq, wk, wv, wo):
    from concourse.bass_utils import run_bass_kernel_spmd

    nc = _get_nc()
    in_maps = make_in_maps(x, wq, wk, wv, wo)
    res = run_bass_kernel_spmd(nc, in_maps, CORE_IDS)
    return gather(res.results)
